# revision 15
# baseline (speedup 1.0000x reference)
"""C2Q attention kernel for Trainium2 (8 NeuronCores, SPMD over batch).

Computes, for inputs similarity [B=32, C=2048, Q=512] f32 and
qencode [B=32, Q=512, H=1024] f32:

    attn = softmax(similarity, axis=-1)
    out  = einsum('bcq,bqh->bch', attn, qencode)

Sharding: data-parallel over batch, 4 batches per core, no collectives.

I/O precision: similarity and qencode are cast to fp16 on the HOST
(before the device DMA) and the output is produced as bf16 and cast
back to f32 on the host. This halves HBM traffic (58.7 -> 29.4 MB per
core), which the profile shows is the limiting resource of the f32
version (DMA engines ~92% occupied, aggregate ~360 GB/s). Numerics:
fp16 sim gives exp() rel err ~2^-11, fp16 matmul operands ~5e-4,
bf16 output ~2e-3 -- all far inside the 2e-2 gate.

Per-core pipeline, per group of 4 C-tiles (128 rows each):
  0.5 MiB batched DMA in -> ACT exp (f16 -> f16) with the softmax
  denominator accumulated for free via accum_out -> PE transpose of the
  exp'd tile to [q, c] layout -> PE matmul contraction over q (fp16)
  -> normalization fused into the PSUM->SBUF copies (ACT & DVE, bf16
  out) -> 2x 0.5 MiB batched DMA out. Software-pipelined three deep.
"""

import numpy as np
import ml_dtypes
from contextlib import ExitStack

import concourse.bass as bass
import concourse.tile as tile
from concourse import bacc, mybir
from concourse.bass_utils import run_bass_kernel_spmd
from concourse.masks import make_identity

B, C, Q, H = 32, 2048, 512, 1024
N_CORES = 8
BPC = B // N_CORES          # batches per core
P = 128                     # partitions
CT = C // P                 # c-tiles per batch
KQ = Q // P                 # q chunks (contraction tiles)
NH = H // 512               # h psum banks per c-tile
GW = 4                      # c-tiles per DMA group
NG = BPC * CT // GW         # total groups per core

F32 = mybir.dt.float32
F16 = mybir.dt.float16
BF16 = mybir.dt.bfloat16

MM_MODE = "fp16"


def build_nc(mm_mode=MM_MODE):
    mm_dt = F16

    nc = bacc.Bacc(None, target_bir_lowering=False)
    sim = nc.dram_tensor("similarity", [BPC, C, Q], F16, kind="ExternalInput")
    qe = nc.dram_tensor("qencode", [BPC, Q, H], F16, kind="ExternalInput")
    out = nc.dram_tensor("out", [BPC, C, H], BF16, kind="ExternalOutput")

    with ExitStack() as ctx:
        tc = ctx.enter_context(tile.TileContext(nc))

        const_pool = ctx.enter_context(tc.tile_pool(name="const", bufs=1))
        ident = const_pool.tile([P, P], mm_dt)
        make_identity(nc, ident[:])

        qe_pool = ctx.enter_context(tc.tile_pool(name="qe", bufs=BPC))
        sim_pool = ctx.enter_context(tc.tile_pool(name="simt", bufs=4))
        expn_pool = ctx.enter_context(tc.tile_pool(name="expn", bufs=GW + 2))
        expT_pool = ctx.enter_context(tc.tile_pool(name="expT", bufs=2 * GW + 2))
        out_pool = ctx.enter_context(tc.tile_pool(name="outsb", bufs=4))
        den_pool = ctx.enter_context(tc.tile_pool(name="den", bufs=3))
        recip_pool = ctx.enter_context(tc.tile_pool(name="recip", bufs=3))
        tr_pool = ctx.enter_context(tc.tile_pool(name="trps", bufs=3, space="PSUM"))
        mm_pool = ctx.enter_context(tc.tile_pool(name="mmps", bufs=5, space="PSUM"))

        qe_tiles = {}

        def load_qe(b):
            qe_t = qe_pool.tile([P, KQ * H], mm_dt, name="qe_t")
            # batch 0 loads chunk-by-chunk so the first matmuls only
            # wait on chunk 0; later batches load in one DMA to keep
            # the SP issue queue light
            nk = KQ if b == 0 else 1
            for k in range(nk):
                w = KQ // nk
                nc.sync.dma_start(
                    qe_t[:, k * w * H:(k + 1) * w * H].rearrange(
                        "p (k h) -> p k h", h=H),
                    qe[b, k * w * P:(k + 1) * w * P, :].rearrange(
                        "(k p) h -> p k h", p=P),
                )
            qe_tiles[b] = qe_t

        def stage_dma(b, g, split=False):
            """Batched 0.5 MiB load of GW c-tiles (natural [c, q] layout).
            The first group loads tile-by-tile so the first exp (and the
            PE pipeline behind it) starts ~4x earlier."""
            sim_t = sim_pool.tile([P, GW * Q], F16, name="sim_t")
            nt = GW if split else 1
            for s in range(nt):
                w = GW // nt
                nc.sync.dma_start(
                    sim_t[:, s * w * Q:(s + 1) * w * Q].rearrange(
                        "p (gg q) -> p gg q", q=Q),
                    sim[b, (g * GW + s * w) * P:(g * GW + (s + 1) * w) * P, :
                        ].rearrange("(gg p) q -> p gg q", p=P),
                )
            return (b, g, sim_t)

        def stage_exp(st):
            """exp on ACT (f16 -> f16) with the softmax denominator
            accumulated on the side; one reciprocal per group on DVE."""
            b, g, sim_t = st
            den = den_pool.tile([P, GW], F32, name="den")
            exps = []
            for t in range(GW):
                e = expn_pool.tile([P, Q], mm_dt, name="expn")
                nc.scalar.activation(
                    e[:], sim_t[:, t * Q:(t + 1) * Q],
                    mybir.ActivationFunctionType.Exp,
                    accum_out=den[:, t:t + 1],
                )
                exps.append(e)
            recip = recip_pool.tile([P, GW], F32, name="recip")
            nc.vector.reciprocal(recip[:], den[:])
            return (b, g, exps, recip)

        def stage_tr(st):
            """PE transpose of the exp'd tiles into [q, c] layout + DVE
            copies PSUM -> SBUF (matmul weights must live in SBUF)."""
            b, g, exps, recip = st
            expTs = []
            for t in range(GW):
                tr = tr_pool.tile([P, Q], mm_dt, name="tr")
                src = exps[t]
                for k in range(KQ):
                    nc.tensor.transpose(
                        tr[:, k * P:(k + 1) * P],
                        src[:, k * P:(k + 1) * P],
                        ident[:],
                    )
                expT = expT_pool.tile([P, Q], mm_dt, name="expT")
                nc.vector.tensor_copy(expT[:], tr[:])
                expTs.append(expT)
            return (b, g, expTs, recip, qe_tiles[b])

        def stage_work(st):
            """Contraction over q on PE, normalization fused into the
            PSUM->SBUF copies (bf16 out), 0.5 MiB stores per half group."""
            b, g, expTs, recip, qe_t = st
            out_sb = out_pool.tile([P, GW * H], BF16, name="out_sb")
            for t in range(GW):
                expT = expTs[t]
                r = recip[:, t:t + 1]
                for h in range(NH):
                    ps = mm_pool.tile([P, 512], F32, name="mm_ps")
                    for k in range(KQ):
                        nc.tensor.matmul(
                            ps[:],
                            expT[:, k * P:(k + 1) * P],
                            qe_t[:, k * H + h * 512: k * H + h * 512 + 512],
                            start=(k == 0),
                            stop=(k == KQ - 1),
                        )
                    o = t * H + h * 512
                    # ~5/16 of the normalize-copies on ACT (which also
                    # runs the exps + accum readouts), the rest on DVE,
                    # so both engines carry ~95us each.
                    idx = (g * GW * NH + 2 * t + h) % 16
                    if idx in (1, 4, 7, 10, 13):
                        nc.scalar.activation(
                            out_sb[:, o:o + 512], ps[:],
                            mybir.ActivationFunctionType.Copy, scale=r,
                        )
                    else:
                        nc.vector.tensor_scalar_mul(out_sb[:, o:o + 512], ps[:], r)
                if t % (GW // 2) == GW // 2 - 1:
                    # store each half-group as soon as its copies land;
                    # alternate the issuing queue (SP / ACT) so neither
                    # serializes the pipeline
                    half = t // (GW // 2)          # 0 or 1
                    hp = GW // 2 * P               # c-rows per half
                    c0 = g * GW * P + half * hp
                    dma_eng = nc.sync if (g + half) % 2 == 0 else nc.scalar
                    dma_eng.dma_start(
                        out[b, c0:c0 + hp, :].rearrange("(gg p) h -> p gg h", p=P),
                        out_sb[:, half * (GW // 2) * H:(half + 1) * (GW // 2) * H
                               ].rearrange("p (gg h) -> p gg h", h=H),
                    )

        # qe for batch 0 first (first matmuls need chunk 0), then the
        # rest up front -- 4 MiB total, lives in SBUF for the whole run.
        load_qe(0)

        # 3-deep software pipeline over groups:
        #   iteration i emits DMA(i), EXP(i-1), WORK(i-2), TR(i-1)
        # WORK is emitted BEFORE TR so the PE's in-order queue runs the
        # (long-ready) matmuls of group i-2 first instead of stalling
        # head-of-line on transposes whose exp just started.
        bg = [(b, g) for b in range(BPC) for g in range(CT // GW)]
        st_dma = st_tr = None
        for i in range(len(bg) + 2):
            new_dma = stage_dma(*bg[i], split=(i == 0 and False)) if i < len(bg) else None
            # stagger the remaining qe batch loads (needed at iterations
            # 4/8/12) so they don't contend with the warmup sim loads
            if i in (1, 3, 5):
                load_qe((i + 1) // 2)
            new_exp = stage_exp(st_dma) if st_dma is not None else None
            if st_tr is not None:
                stage_work(st_tr)
            new_tr = stage_tr(new_exp) if new_exp is not None else None
            st_dma, st_tr = new_dma, new_tr

    nc.finalize()
    return nc


_NC_CACHE = {}


def _get_nc(mode=MM_MODE):
    if mode not in _NC_CACHE:
        _NC_CACHE[mode] = build_nc(mode)
    return _NC_CACHE[mode]


def run(similarity, qencode, mode=MM_MODE, **spmd_kwargs):
    nc = _get_nc(mode)
    similarity = np.ascontiguousarray(similarity).astype(np.float16)
    qencode = np.ascontiguousarray(qencode).astype(np.float16)
    in_maps = [
        {
            "similarity": similarity[i * BPC:(i + 1) * BPC],
            "qencode": qencode[i * BPC:(i + 1) * BPC],
        }
        for i in range(N_CORES)
    ]
    res = run_bass_kernel_spmd(nc, in_maps, core_ids=list(range(N_CORES)), **spmd_kwargs)
    out = np.concatenate([res.results[i]["out"] for i in range(N_CORES)], axis=0)
    return out.astype(np.float32), res


def kernel(similarity, qencode):
    out, _ = run(similarity, qencode)
    return out


# revision 16
# speedup vs baseline: 1.0738x; 1.0738x over previous
"""C2Q attention kernel for Trainium2 (8 NeuronCores, SPMD over batch).

Computes, for inputs similarity [B=32, C=2048, Q=512] f32 and
qencode [B=32, Q=512, H=1024] f32:

    attn = softmax(similarity, axis=-1)
    out  = einsum('bcq,bqh->bch', attn, qencode)

Sharding: data-parallel over batch, 4 batches per core, no collectives.

I/O precision: similarity and qencode are cast to fp16 on the HOST
(before the device DMA) and the output is produced as bf16 and cast
back to f32 on the host. This halves HBM traffic (58.7 -> 29.4 MB per
core), which the profile shows is the limiting resource of the f32
version (DMA engines ~92% occupied, aggregate ~360 GB/s). Numerics:
fp16 sim gives exp() rel err ~2^-11, fp16 matmul operands ~5e-4,
bf16 output ~2e-3 -- all far inside the 2e-2 gate.

Per-core pipeline, per group of 4 C-tiles (128 rows each):
  0.5 MiB batched DMA in -> ACT exp (f16 -> f16) with the softmax
  denominator accumulated for free via accum_out -> PE transpose of the
  exp'd tile to [q, c] layout -> PE matmul contraction over q (fp16)
  -> normalization fused into the PSUM->SBUF copies (ACT & DVE, bf16
  out) -> 2x 0.5 MiB batched DMA out. Software-pipelined three deep.
"""

import numpy as np
import ml_dtypes
from contextlib import ExitStack

import concourse.bass as bass
import concourse.tile as tile
from concourse import bacc, mybir
from concourse.bass_utils import run_bass_kernel_spmd
from concourse.masks import make_identity

B, C, Q, H = 32, 2048, 512, 1024
N_CORES = 8
BPC = B // N_CORES          # batches per core
P = 128                     # partitions
CT = C // P                 # c-tiles per batch
KQ = Q // P                 # q chunks (contraction tiles)
NH = H // 512               # h psum banks per c-tile
GW = 4                      # c-tiles per DMA group
NG = BPC * CT // GW         # total groups per core

F32 = mybir.dt.float32
F16 = mybir.dt.float16
BF16 = mybir.dt.bfloat16

MM_MODE = "fp16"


def build_nc(mm_mode=MM_MODE):
    mm_dt = F16

    nc = bacc.Bacc(None, target_bir_lowering=False)
    sim = nc.dram_tensor("similarity", [BPC, C, Q], F16, kind="ExternalInput")
    qe = nc.dram_tensor("qencode", [BPC, Q, H], F16, kind="ExternalInput")
    out = nc.dram_tensor("out", [BPC, C, H], BF16, kind="ExternalOutput")

    with ExitStack() as ctx:
        tc = ctx.enter_context(tile.TileContext(nc))

        const_pool = ctx.enter_context(tc.tile_pool(name="const", bufs=1))
        ident = const_pool.tile([P, P], mm_dt)
        make_identity(nc, ident[:])

        qe_pool = ctx.enter_context(tc.tile_pool(name="qe", bufs=BPC))
        sim_pool = ctx.enter_context(tc.tile_pool(name="simt", bufs=4))
        expn_pool = ctx.enter_context(tc.tile_pool(name="expn", bufs=GW + 2))
        expT_pool = ctx.enter_context(tc.tile_pool(name="expT", bufs=2 * GW + 2))
        out_pool = ctx.enter_context(tc.tile_pool(name="outsb", bufs=4))
        den_pool = ctx.enter_context(tc.tile_pool(name="den", bufs=3))
        recip_pool = ctx.enter_context(tc.tile_pool(name="recip", bufs=3))
        tr_pool = ctx.enter_context(tc.tile_pool(name="trps", bufs=3, space="PSUM"))
        mm_pool = ctx.enter_context(tc.tile_pool(name="mmps", bufs=5, space="PSUM"))

        qe_tiles = {}

        def load_qe(b):
            qe_t = qe_pool.tile([P, KQ * H], mm_dt, name="qe_t")
            # batch 0 loads chunk-by-chunk so the first matmuls only
            # wait on chunk 0; later batches load in one DMA to keep
            # the SP issue queue light
            nk = KQ if b == 0 else 1
            for k in range(nk):
                w = KQ // nk
                nc.sync.dma_start(
                    qe_t[:, k * w * H:(k + 1) * w * H].rearrange(
                        "p (k h) -> p k h", h=H),
                    qe[b, k * w * P:(k + 1) * w * P, :].rearrange(
                        "(k p) h -> p k h", p=P),
                )
            qe_tiles[b] = qe_t

        def stage_dma(b, g, split=False):
            """Batched 0.5 MiB load of GW c-tiles (natural [c, q] layout).
            The first group loads tile-by-tile so the first exp (and the
            PE pipeline behind it) starts ~4x earlier."""
            sim_t = sim_pool.tile([P, GW * Q], F16, name="sim_t")
            nt = GW if split else 1
            for s in range(nt):
                w = GW // nt
                nc.sync.dma_start(
                    sim_t[:, s * w * Q:(s + 1) * w * Q].rearrange(
                        "p (gg q) -> p gg q", q=Q),
                    sim[b, (g * GW + s * w) * P:(g * GW + (s + 1) * w) * P, :
                        ].rearrange("(gg p) q -> p gg q", p=P),
                )
            return (b, g, sim_t)

        def stage_exp(st):
            """exp on ACT (f16 -> f16) with the softmax denominator
            accumulated on the side; one reciprocal per group on DVE."""
            b, g, sim_t = st
            den = den_pool.tile([P, GW], F32, name="den")
            exps = []
            for t in range(GW):
                e = expn_pool.tile([P, Q], mm_dt, name="expn")
                nc.scalar.activation(
                    e[:], sim_t[:, t * Q:(t + 1) * Q],
                    mybir.ActivationFunctionType.Exp,
                    accum_out=den[:, t:t + 1],
                )
                exps.append(e)
            recip = recip_pool.tile([P, GW], F32, name="recip")
            nc.vector.reciprocal(recip[:], den[:])
            return (b, g, exps, recip)

        def stage_tr(st):
            """PE transpose of the exp'd tiles into [q, c] layout + DVE
            copies PSUM -> SBUF (matmul weights must live in SBUF)."""
            b, g, exps, recip = st
            expTs = []
            for t in range(GW):
                tr = tr_pool.tile([P, Q], mm_dt, name="tr")
                src = exps[t]
                for k in range(KQ):
                    nc.tensor.transpose(
                        tr[:, k * P:(k + 1) * P],
                        src[:, k * P:(k + 1) * P],
                        ident[:],
                    )
                expT = expT_pool.tile([P, Q], mm_dt, name="expT")
                nc.vector.tensor_copy(expT[:], tr[:])
                expTs.append(expT)
            return (b, g, expTs, recip, qe_tiles[b])

        def stage_work(st):
            """Contraction over q on PE, normalization fused into the
            PSUM->SBUF copies (bf16 out), 0.5 MiB stores per half group."""
            b, g, expTs, recip, qe_t = st
            out_sb = out_pool.tile([P, GW * H], BF16, name="out_sb")
            for t in range(GW):
                expT = expTs[t]
                r = recip[:, t:t + 1]
                for h in range(NH):
                    ps = mm_pool.tile([P, 512], F32, name="mm_ps")
                    for k in range(KQ):
                        nc.tensor.matmul(
                            ps[:],
                            expT[:, k * P:(k + 1) * P],
                            qe_t[:, k * H + h * 512: k * H + h * 512 + 512],
                            start=(k == 0),
                            stop=(k == KQ - 1),
                        )
                    o = t * H + h * 512
                    # ~5/16 of the normalize-copies on ACT (which also
                    # runs the exps + accum readouts), the rest on DVE,
                    # so both engines carry ~95us each.
                    idx = (g * GW * NH + 2 * t + h) % 16
                    if idx in (1, 4, 7, 10, 13):
                        nc.scalar.activation(
                            out_sb[:, o:o + 512], ps[:],
                            mybir.ActivationFunctionType.Copy, scale=r,
                        )
                    else:
                        nc.vector.tensor_scalar_mul(out_sb[:, o:o + 512], ps[:], r)
                if t % (GW // 2) == GW // 2 - 1:
                    # store each half-group as soon as its copies land;
                    # alternate the issuing queue (SP / ACT) so neither
                    # serializes the pipeline
                    half = t // (GW // 2)          # 0 or 1
                    hp = GW // 2 * P               # c-rows per half
                    c0 = g * GW * P + half * hp
                    dma_eng = nc.sync if (g + half) % 2 == 0 else nc.scalar
                    dma_eng.dma_start(
                        out[b, c0:c0 + hp, :].rearrange("(gg p) h -> p gg h", p=P),
                        out_sb[:, half * (GW // 2) * H:(half + 1) * (GW // 2) * H
                               ].rearrange("p (gg h) -> p gg h", h=H),
                    )

        # Pre-warm the ACT Exp table on a dummy tile so the one-time
        # ~1.3us table load happens while the first sim DMA is in
        # flight, not after it.
        warm = den_pool.tile([P, 1], F16, name="warm")
        nc.scalar.activation(warm[:], ident[:, 0:1],
                             mybir.ActivationFunctionType.Exp)

        # 3-deep software pipeline over groups:
        #   iteration i emits DMA(i), EXP(i-1), TR(i-1), WORK(i-2).
        # The FIRST sim group is issued before any qe bytes so the exp/
        # transpose pipeline starts on a full-bandwidth 0.5 MiB load;
        # qe batches are staggered behind it (first matmuls only need
        # qe chunk 0, which lands during the first exp+transpose).
        bg = [(b, g) for b in range(BPC) for g in range(CT // GW)]
        st_dma = st_tr = None
        for i in range(len(bg) + 2):
            new_dma = stage_dma(*bg[i]) if i < len(bg) else None
            if i in (0, 2, 4, 6):
                load_qe(i // 2)
            new_exp = stage_exp(st_dma) if st_dma is not None else None
            new_tr = stage_tr(new_exp) if new_exp is not None else None
            if st_tr is not None:
                stage_work(st_tr)
            st_dma, st_tr = new_dma, new_tr

    nc.finalize()
    return nc


_NC_CACHE = {}


def _get_nc(mode=MM_MODE):
    if mode not in _NC_CACHE:
        _NC_CACHE[mode] = build_nc(mode)
    return _NC_CACHE[mode]


def run(similarity, qencode, mode=MM_MODE, **spmd_kwargs):
    nc = _get_nc(mode)
    similarity = np.ascontiguousarray(similarity).astype(np.float16)
    qencode = np.ascontiguousarray(qencode).astype(np.float16)
    in_maps = [
        {
            "similarity": similarity[i * BPC:(i + 1) * BPC],
            "qencode": qencode[i * BPC:(i + 1) * BPC],
        }
        for i in range(N_CORES)
    ]
    res = run_bass_kernel_spmd(nc, in_maps, core_ids=list(range(N_CORES)), **spmd_kwargs)
    out = np.concatenate([res.results[i]["out"] for i in range(N_CORES)], axis=0)
    return out.astype(np.float32), res


def kernel(similarity, qencode):
    out, _ = run(similarity, qencode)
    return out
